# revision 11
# baseline (speedup 1.0000x reference)
"""Trainium2 Bass kernel for nn_DecoderLayer (masked cross-attn + self-attn + FFN).

Sharding: pure data-parallel over batch B=8 across the 8 NeuronCores (one
batch element per core). No collectives. Weights are broadcast to every core
via its input map; LayerNorm gains/betas are folded into the projection
weights host-side (weight-sized preprocessing only).
"""

import numpy as np
import ml_dtypes
from contextlib import ExitStack

import concourse.bass as bass
import concourse.tile as tile
from concourse import mybir
from concourse.masks import make_identity
from concourse.bass_utils import run_bass_kernel_spmd

F32 = mybir.dt.float32
BF16 = mybir.dt.bfloat16
I32 = mybir.dt.int32
AF = mybir.ActivationFunctionType

B, NQ, HWL, C, FF, NH, D = 8, 100, 4096, 512, 2048, 8, 64
CT = C // 128    # 4 c-in tiles
MT = HWL // 128  # 32 m tiles
FT = FF // 128   # 16 ff tiles
EPS = 1e-5
MMDT = BF16      # matmul operand dtype
NPDT = ml_dtypes.bfloat16


def _legalize_single_wait(nc):
    """This toolchain's walrus build accepts at most ONE sem wait per
    instruction; Tile's final drain (and occasionally other instructions)
    carries several. Hoist extra waits onto dedicated NoOps placed
    immediately before the instruction on the same engine queue."""
    uid = 0
    for f in nc.m.functions:
        for bb in f.blocks:
            new = []
            for inst in bb.instructions:
                si = inst.sync_info
                if si is not None and si.on_wait and len(si.on_wait) > 1:
                    waits = list(si.on_wait)
                    for w in waits[:-1]:
                        uid += 1
                        new.append(mybir.InstNoOp(
                            name=f"legw{uid}",
                            engine=inst.engine,
                            sync_info=mybir.SyncInfo(on_wait=[w], on_update=[]),
                            bass_nofuse=True,
                        ))
                    si.on_wait = [waits[-1]]
                new.append(inst)
            bb.instructions[:] = new


def build_nc(legalize=True):
    nc = bass.Bass()

    q_d = nc.dram_tensor('q', [NQ, C], F32, kind='ExternalInput')
    kv_d = nc.dram_tensor('kv', [HWL, C], F32, kind='ExternalInput')
    mask_d = nc.dram_tensor('mask', [NQ, HWL], I32, kind='ExternalInput')
    wq_d = nc.dram_tensor('wq', [C, C], MMDT, kind='ExternalInput')
    wk_d = nc.dram_tensor('wk', [C, C], MMDT, kind='ExternalInput')
    wv_d = nc.dram_tensor('wv', [C, C], MMDT, kind='ExternalInput')
    wo_d = nc.dram_tensor('wo', [C, C], MMDT, kind='ExternalInput')
    cq_d = nc.dram_tensor('cq', [C], F32, kind='ExternalInput')
    ck_d = nc.dram_tensor('ck', [C], F32, kind='ExternalInput')
    cv_d = nc.dram_tensor('cv', [C], F32, kind='ExternalInput')
    bo_d = nc.dram_tensor('bo', [C], F32, kind='ExternalInput')
    swq_d = nc.dram_tensor('swq', [C, C], MMDT, kind='ExternalInput')
    swk_d = nc.dram_tensor('swk', [C, C], MMDT, kind='ExternalInput')
    swv_d = nc.dram_tensor('swv', [C, C], MMDT, kind='ExternalInput')
    swo_d = nc.dram_tensor('swo', [C, C], MMDT, kind='ExternalInput')
    scq_d = nc.dram_tensor('scq', [C], F32, kind='ExternalInput')
    sck_d = nc.dram_tensor('sck', [C], F32, kind='ExternalInput')
    scv_d = nc.dram_tensor('scv', [C], F32, kind='ExternalInput')
    sbo_d = nc.dram_tensor('sbo', [C], F32, kind='ExternalInput')
    w1_d = nc.dram_tensor('w1', [C, FF], MMDT, kind='ExternalInput')
    c1_d = nc.dram_tensor('c1', [FF], F32, kind='ExternalInput')
    w2_d = nc.dram_tensor('w2', [FF, C], MMDT, kind='ExternalInput')
    b2_d = nc.dram_tensor('b2', [C], F32, kind='ExternalInput')
    out_d = nc.dram_tensor('out', [NQ, C], F32, kind='ExternalOutput')

    def bcast(ap, p=128):
        # broadcast a 1-D dram vector across p partitions
        return bass.AP(tensor=ap.tensor, offset=ap.offset, ap=[[0, p]] + list(ap.ap))

    with tile.TileContext(nc) as tc, ExitStack() as ctx:
        const = ctx.enter_context(tc.tile_pool(name='const', bufs=1))
        wpool = ctx.enter_context(tc.tile_pool(name='weights', bufs=1))
        persist = ctx.enter_context(tc.tile_pool(name='persist', bufs=1))

        ident_mm = const.tile([128, 128], MMDT)
        make_identity(nc, ident_mm)
        ident_f32 = const.tile([128, 128], F32)
        make_identity(nc, ident_f32)
        eps_t = const.tile([128, 1], F32)
        nc.vector.memset(eps_t[:], EPS)

        # ---- weights into SBUF ----
        def load_w(d):  # [C, C] -> [128, CT, C]
            t = wpool.tile([128, CT, C], MMDT, tag=f'w_{d.name}')
            nc.sync.dma_start(out=t[:], in_=d.rearrange("(a p) c -> p a c", p=128))
            return t

        def load_w_h(d):  # [C, C] -> [64, NH, C]  (row = within-head dim)
            t = wpool.tile([64, NH, C], MMDT, tag=f'wh_{d.name}')
            nc.sync.dma_start(out=t[:], in_=d.rearrange("(h dd) c -> dd h c", dd=64))
            return t

        def load_b(d, n):  # [n*128] -> [128, n]
            t = wpool.tile([128, n], F32, tag=f'b_{d.name}')
            nc.sync.dma_start(out=t[:], in_=d.rearrange("(a p) -> p a", p=128))
            return t

        def load_b_h(d):  # [C] -> [64, NH]
            t = wpool.tile([64, NH], F32, tag=f'bh_{d.name}')
            nc.sync.dma_start(out=t[:], in_=d.rearrange("(h dd) -> dd h", dd=64))
            return t

        def load_bc(d):  # [C] -> [128, C] partition-broadcast
            t = wpool.tile([128, C], F32, tag=f'bc_{d.name}')
            nc.sync.dma_start(out=t[:], in_=bcast(d[:]))
            return t

        wq_s, wk_s, wv_s = load_w(wq_d), load_w(wk_d), load_w(wv_d)
        wo_s = load_w_h(wo_d)
        cq_s, ck_s = load_b(cq_d, CT), load_b(ck_d, CT)
        cv_bc = load_bc(cv_d)
        bo_bc = load_bc(bo_d)
        swq_s, swk_s, swv_s = load_w(swq_d), load_w(swk_d), load_w(swv_d)
        swo_s = load_w_h(swo_d)
        scq_s, sck_s = load_b(scq_d, CT), load_b(sck_d, CT)
        scv_s = load_b_h(scv_d)
        sbo_bc = load_bc(sbo_d)
        c1_s = load_b(c1_d, FT)
        b2_bc = load_bc(b2_d)

        # persistent activations
        kpT_s = persist.tile([128, CT, HWL], MMDT)      # K^T proj [c_out, m]
        vp_s = persist.tile([128, MT, NH, D + 1], MMDT)  # V rows + ones col
        effT_s = persist.tile([128, MT, NQ], MMDT)       # mask^T (eff)
        q_sb = persist.tile([NQ, C], F32)                # original q (residual)
        q2_sb = persist.tile([NQ, C], F32)
        q3_sb = persist.tile([NQ, C], F32)
        qpT_s = persist.tile([128, CT, NQ], MMDT)
        sqpT_s = persist.tile([128, CT, NQ], MMDT)
        skpT_s = persist.tile([128, CT, NQ], MMDT)
        svp_s = persist.tile([NQ, NH, D], MMDT)
        hT_s = persist.tile([128, FT, NQ], MMDT)

        nc.vector.memset(vp_s[:, :, :, D:D + 1], 1.0)

        small = ctx.enter_context(tc.tile_pool(name='small', bufs=4))
        tppool = ctx.enter_context(tc.tile_pool(name='tp_psum', bufs=2, space='PSUM'))

        def ln_norm(src_ap, p, out_dt=MMDT):
            """LayerNorm (stats+normalize only; affine folded into weights)."""
            stats = small.tile([p, 6], F32, tag='ln_stats')
            nc.vector.bn_stats(out=stats[:], in_=src_ap)
            mv = small.tile([p, 2], F32, tag='ln_mv')
            nc.vector.bn_aggr(out=mv[:], in_=stats[:])
            rstd = small.tile([p, 1], F32, tag='ln_rstd')
            nc.scalar.activation(rstd[:], mv[:, 1:2], AF.Sqrt, bias=eps_t[:p])
            nc.vector.reciprocal(out=rstd[:], in_=rstd[:])
            o = small.tile([p, C], out_dt, tag='ln_out')
            nc.vector.tensor_scalar(
                out=o[:], in0=src_ap, scalar1=mv[:, 0:1], scalar2=rstd[:],
                op0=mybir.AluOpType.subtract, op1=mybir.AluOpType.mult)
            return o

        def transpose_cols(src, p, ncols, dst_fn, dt=MMDT):
            """PE-transpose [p, ncols*128] -> ncols blocks of [128, p]."""
            ident = ident_mm if dt == MMDT else ident_f32
            for j in range(ncols):
                pt = tppool.tile([128, 128], dt,
                                 tag='tp_f32' if dt == F32 else 'tp_bf16')
                nc.tensor.transpose(pt[:, :p], src[:p, j * 128:(j + 1) * 128],
                                    ident[:p, :p])
                nc.vector.tensor_copy(out=dst_fn(j), in_=pt[:, :p])

        # ================= mask prep =================
        with tc.tile_pool(name='mask_row', bufs=1) as mrow:
            m_i32 = mrow.tile([NQ, HWL], I32)
            nc.sync.dma_start(out=m_i32[:], in_=mask_d[:])
            m_bf = mrow.tile([NQ, HWL], MMDT)
            nc.vector.tensor_copy(out=m_bf[:], in_=m_i32[:])
            rany = small.tile([NQ, 1], F32, tag='rany')
            nc.vector.reduce_max(out=rany[:], in_=m_bf[:], axis=mybir.AxisListType.X)
            gfb = small.tile([NQ, 1], F32, tag='gfb')
            nc.scalar.activation(gfb[:], rany[:], AF.Identity, bias=1.0, scale=-1.0)
            nc.vector.tensor_scalar_max(out=m_bf[:], in0=m_bf[:], scalar1=gfb[:])
            transpose_cols(m_bf, NQ, MT, lambda j: effT_s[:, j, :])

        # ================= kv LN + transpose + K/V proj =================
        with tc.tile_pool(name='kvpool', bufs=3) as kvpool, \
             tc.tile_pool(name='kvnT', bufs=1) as kvnTp, \
             tc.tile_pool(name='proj_psum', bufs=2, space='PSUM') as pjp:
            kvnT = kvnTp.tile([128, CT, HWL], MMDT)
            for mi in range(MT):
                kvt = kvpool.tile([128, C], F32, tag='kv_in')
                nc.sync.dma_start(out=kvt[:], in_=kv_d[mi * 128:(mi + 1) * 128, :])
                kvn = ln_norm(kvt[:], 128)
                transpose_cols(kvn, 128, CT, lambda j, mi=mi: kvnT[:, j, mi * 128:(mi + 1) * 128])
            # K^T projection: [c_out, m]
            for ct2 in range(CT):
                for mj in range(HWL // 512):
                    pk = pjp.tile([128, 512], F32, tag='pk')
                    for ci in range(CT):
                        nc.tensor.matmul(
                            pk[:], wk_s[:, ci, ct2 * 128:(ct2 + 1) * 128],
                            kvnT[:, ci, mj * 512:(mj + 1) * 512],
                            start=(ci == 0), stop=(ci == CT - 1))
                    nc.scalar.activation(
                        kpT_s[:, ct2, mj * 512:(mj + 1) * 512], pk[:],
                        AF.Identity, bias=ck_s[:, ct2:ct2 + 1])
            # V projection: rows [m, c_out]
            for mi in range(MT):
                pv = pjp.tile([128, 512], F32, tag='pv')
                for ci in range(CT):
                    nc.tensor.matmul(
                        pv[:], kvnT[:, ci, mi * 128:(mi + 1) * 128],
                        wv_s[:, ci, :], start=(ci == 0), stop=(ci == CT - 1))
                # add cv happens via the ones-column correction later
                nc.scalar.activation(
                    vp_s[:, mi, :, 0:D],
                    pv[:].rearrange("p (h d) -> p h d", h=NH), AF.Copy)

        # ================= q LN + Q proj =================
        nc.sync.dma_start(out=q_sb[:], in_=q_d[:])
        qn = ln_norm(q_sb[:], NQ)
        qnT = persist.tile([128, CT, NQ], MMDT)
        transpose_cols(qn, NQ, CT, lambda j: qnT[:, j, :])
        with tc.tile_pool(name='qp_psum', bufs=2, space='PSUM') as qpp:
            for ct2 in range(CT):
                pq = qpp.tile([128, 128], F32, tag='pq')
                for ci in range(CT):
                    nc.tensor.matmul(
                        pq[:, :NQ], wq_s[:, ci, ct2 * 128:(ct2 + 1) * 128],
                        qnT[:, ci, :], start=(ci == 0), stop=(ci == CT - 1))
                nc.scalar.activation(qpT_s[:, ct2, :], pq[:, :NQ], AF.Identity,
                                     bias=cq_s[:, ct2:ct2 + 1])

        # ================= cross-attention heads =================
        with tc.tile_pool(name='xa_psum', bufs=2, space='PSUM') as xap, \
             tc.tile_pool(name='u_psum', bufs=1, space='PSUM') as upp, \
             tc.tile_pool(name='xa_sbuf', bufs=3) as xas:
            u_ps = upp.tile([NQ, C], F32, tag='u')
            for h in range(NH):
                ht, po = h // 2, (h % 2) * 64
                a_u = xas.tile([128, MT, NQ], MMDT, tag='a_u')
                for qj in range(8):
                    ps = xap.tile([128, 4, 128], F32, tag='ps')
                    for j in range(4):
                        mj = qj * 4 + j
                        nc.tensor.matmul(
                            ps[:, j, :NQ],
                            kpT_s[po:po + 64, ht, mj * 128:(mj + 1) * 128],
                            qpT_s[po:po + 64, ht, :], start=True, stop=True)
                    a_e = xas.tile([128, 4, NQ], MMDT, tag='a_e')
                    nc.scalar.activation(a_e[:], ps[:, :, :NQ], AF.Exp)
                    nc.vector.tensor_mul(
                        out=a_u[:, qj * 4:(qj + 1) * 4, :], in0=a_e[:],
                        in1=effT_s[:, qj * 4:(qj + 1) * 4, :])
                oa = upp.tile([D + 1, NQ], F32, tag='oa')
                for mi in range(MT):
                    nc.tensor.matmul(
                        oa[:], vp_s[:, mi, h, :], a_u[:, mi, :],
                        start=(mi == 0), stop=(mi == MT - 1))
                oa_sb = xas.tile([D + 1, NQ], F32, tag='oa_sb')
                nc.scalar.activation(oa_sb[:], oa[:], AF.Copy)
                ot = tppool.tile([NQ, D + 1], F32, tag='tp_f32')
                nc.tensor.transpose(ot[:], oa_sb[:], ident_f32[:D + 1, :D + 1])
                sums = xas.tile([NQ, 1], F32, tag='sums')
                nc.scalar.activation(sums[:], ot[:, D:D + 1], AF.Copy)
                r_sb = xas.tile([NQ, 1], F32, tag='r_sb')
                nc.vector.reciprocal(out=r_sb[:], in_=sums[:])
                tmp = xas.tile([NQ, D], F32, tag='tmp')
                nc.vector.tensor_scalar_mul(
                    out=tmp[:], in0=cv_bc[:NQ, h * 64:(h + 1) * 64], scalar1=sums[:])
                ocorr = xas.tile([NQ, D], F32, tag='ocorr')
                nc.vector.tensor_add(out=ocorr[:], in0=ot[:, 0:D], in1=tmp[:])
                o_n = xas.tile([NQ, D], MMDT, tag='o_n')
                nc.vector.tensor_scalar_mul(out=o_n[:], in0=ocorr[:], scalar1=r_sb[:])
                ot2 = tppool.tile([D, NQ], MMDT, tag='tp_bf16')
                nc.tensor.transpose(ot2[:], o_n[:], ident_mm[:NQ, :NQ])
                oT_sb = xas.tile([D, NQ], MMDT, tag='oT_sb')
                nc.scalar.activation(oT_sb[:], ot2[:], AF.Copy)
                nc.tensor.matmul(u_ps[:], oT_sb[:], wo_s[:, h, :],
                                 start=(h == 0), stop=(h == NH - 1))
            nc.vector.tensor_add(out=q2_sb[:], in0=u_ps[:], in1=q_sb[:])
            nc.vector.tensor_add(out=q2_sb[:], in0=q2_sb[:], in1=bo_bc[:NQ, :])

        # ================= self-attention =================
        with tc.tile_pool(name='sa_psum', bufs=2, space='PSUM') as sap, \
             tc.tile_pool(name='u2_psum', bufs=1, space='PSUM') as up2, \
             tc.tile_pool(name='sa_sbuf', bufs=3) as sas:
            qn1 = ln_norm(q2_sb[:], NQ)
            qn1T = persist.tile([128, CT, NQ], MMDT)
            transpose_cols(qn1, NQ, CT, lambda j: qn1T[:, j, :])
            for ct2 in range(CT):
                pq = sap.tile([128, 128], F32, tag='sa_ps')
                for ci in range(CT):
                    nc.tensor.matmul(
                        pq[:, :NQ], swq_s[:, ci, ct2 * 128:(ct2 + 1) * 128],
                        qn1T[:, ci, :], start=(ci == 0), stop=(ci == CT - 1))
                nc.scalar.activation(sqpT_s[:, ct2, :], pq[:, :NQ], AF.Identity,
                                     bias=scq_s[:, ct2:ct2 + 1])
                pk = sap.tile([128, 128], F32, tag='sa_ps')
                for ci in range(CT):
                    nc.tensor.matmul(
                        pk[:, :NQ], swk_s[:, ci, ct2 * 128:(ct2 + 1) * 128],
                        qn1T[:, ci, :], start=(ci == 0), stop=(ci == CT - 1))
                nc.scalar.activation(skpT_s[:, ct2, :], pk[:, :NQ], AF.Identity,
                                     bias=sck_s[:, ct2:ct2 + 1])
            pv = up2.tile([NQ, C], F32, tag='spv')
            for ci in range(CT):
                nc.tensor.matmul(pv[:], qn1T[:, ci, :], swv_s[:, ci, :],
                                 start=(ci == 0), stop=(ci == CT - 1))
            nc.scalar.activation(svp_s[:],
                                 pv[:].rearrange("p (h d) -> p h d", h=NH), AF.Copy)
            u2_ps = up2.tile([NQ, C], F32, tag='u2')
            for h in range(NH):
                ht, po = h // 2, (h % 2) * 64
                pss = sap.tile([NQ, 128], F32, tag='sa_ps')
                nc.tensor.matmul(pss[:, :NQ], sqpT_s[po:po + 64, ht, :],
                                 skpT_s[po:po + 64, ht, :], start=True, stop=True)
                sa_a = sas.tile([NQ, NQ], MMDT, tag='sa_a')
                ssum = sas.tile([NQ, 1], F32, tag='ssum')
                nc.scalar.activation(sa_a[:], pss[:, :NQ], AF.Exp, accum_out=ssum[:])
                sr = sas.tile([NQ, 1], F32, tag='sr')
                nc.vector.reciprocal(out=sr[:], in_=ssum[:])
                sa_an = sas.tile([NQ, NQ], MMDT, tag='sa_an')
                nc.vector.tensor_scalar_mul(out=sa_an[:], in0=sa_a[:], scalar1=sr[:])
                pat = tppool.tile([NQ, NQ], MMDT, tag='tp_bf16')
                nc.tensor.transpose(pat[:], sa_an[:], ident_mm[:NQ, :NQ])
                aT_sb = sas.tile([NQ, NQ], MMDT, tag='aT_sb')
                nc.scalar.activation(aT_sb[:], pat[:], AF.Copy)
                pso = sap.tile([D, NQ], F32, tag='sa_ps')
                nc.tensor.matmul(pso[:], svp_s[:, h, :], aT_sb[:],
                                 start=True, stop=True)
                soT = sas.tile([D, NQ], MMDT, tag='soT')
                nc.scalar.activation(soT[:], pso[:], AF.Identity,
                                     bias=scv_s[:, h:h + 1])
                nc.tensor.matmul(u2_ps[:], soT[:], swo_s[:, h, :],
                                 start=(h == 0), stop=(h == NH - 1))
            nc.vector.tensor_add(out=q3_sb[:], in0=u2_ps[:], in1=q2_sb[:])
            nc.vector.tensor_add(out=q3_sb[:], in0=q3_sb[:], in1=sbo_bc[:NQ, :])

        # ================= FFN =================
        with tc.tile_pool(name='ff_psum', bufs=3, space='PSUM') as ffp, \
             tc.tile_pool(name='pf_psum', bufs=1, space='PSUM') as pfp, \
             tc.tile_pool(name='ffw', bufs=1) as ffw, \
             tc.tile_pool(name='ff_sbuf', bufs=2) as ffs:
            w1_s = ffw.tile([128, CT, FF], MMDT)
            nc.sync.dma_start(out=w1_s[:], in_=w1_d.rearrange("(a p) c -> p a c", p=128))
            w2_s = ffw.tile([128, FT, C], MMDT)
            nc.sync.dma_start(out=w2_s[:], in_=w2_d.rearrange("(a p) c -> p a c", p=128))
            qn2 = ln_norm(q3_sb[:], NQ)
            qn2T = persist.tile([128, CT, NQ], MMDT)
            transpose_cols(qn2, NQ, CT, lambda j: qn2T[:, j, :])
            for ft in range(FT):
                ph = ffp.tile([128, 128], F32, tag='ph')
                for ci in range(CT):
                    nc.tensor.matmul(
                        ph[:, :NQ], w1_s[:, ci, ft * 128:(ft + 1) * 128],
                        qn2T[:, ci, :], start=(ci == 0), stop=(ci == CT - 1))
                nc.scalar.activation(hT_s[:, ft, :], ph[:, :NQ], AF.Relu,
                                     bias=c1_s[:, ft:ft + 1])
            pf = pfp.tile([NQ, C], F32, tag='pf')
            for fi in range(FT):
                nc.tensor.matmul(pf[:], hT_s[:, fi, :], w2_s[:, fi, :],
                                 start=(fi == 0), stop=(fi == FT - 1))
            fin = ffs.tile([NQ, C], F32, tag='fin')
            nc.vector.tensor_add(out=fin[:], in0=pf[:], in1=q3_sb[:])
            nc.vector.tensor_add(out=fin[:], in0=fin[:], in1=b2_bc[:NQ, :])
            nc.sync.dma_start(out=out_d[:], in_=fin[:])

    if legalize:
        _legalize_single_wait(nc)
    return nc


def _prep_inputs(inputs):
    """Host-side weight preprocessing: fold LN affine into projections,
    pre-scale Q by 1/sqrt(d), cast matmul operands to bf16."""
    f = {k: np.asarray(v, np.float32) for k, v in inputs.items()}
    s = 1.0 / np.sqrt(D)

    Wq, Wk, Wv = np.split(f['xa_Wqkv'], 3, axis=1)
    bq, bk, bv = np.split(f['xa_bqkv'], 3)
    wq = (f['xa_gq'][:, None] * Wq) * s
    cq = (f['xa_bq_ln'] @ Wq + bq) * s
    wk = f['xa_gkv'][:, None] * Wk
    ck = f['xa_bkv_ln'] @ Wk + bk
    wv = f['xa_gkv'][:, None] * Wv
    cv = f['xa_bkv_ln'] @ Wv + bv

    sWq, sWk, sWv = np.split(f['sa_Wqkv'], 3, axis=1)
    sbq, sbk, sbv = np.split(f['sa_bqkv'], 3)
    swq = (f['n1_g'][:, None] * sWq) * s
    scq = (f['n1_b'] @ sWq + sbq) * s
    swk = f['n1_g'][:, None] * sWk
    sck = f['n1_b'] @ sWk + sbk
    swv = f['n1_g'][:, None] * sWv
    scv = f['n1_b'] @ sWv + sbv

    w1 = f['n2_g'][:, None] * f['ff_W1']
    c1 = f['n2_b'] @ f['ff_W1'] + f['ff_b1']

    shared = {
        'wq': wq.astype(NPDT), 'wk': wk.astype(NPDT), 'wv': wv.astype(NPDT),
        'wo': f['xa_Wo'].astype(NPDT),
        'cq': cq, 'ck': ck, 'cv': cv, 'bo': f['xa_bo'],
        'swq': swq.astype(NPDT), 'swk': swk.astype(NPDT),
        'swv': swv.astype(NPDT), 'swo': f['sa_Wo'].astype(NPDT),
        'scq': scq, 'sck': sck, 'scv': scv, 'sbo': f['sa_bo'],
        'w1': w1.astype(NPDT), 'c1': c1,
        'w2': f['ff_W2'].astype(NPDT), 'b2': f['ff_b2'],
    }
    q = np.asarray(inputs['q'], np.float32)
    kv = np.asarray(inputs['kv'], np.float32)
    mask = np.ascontiguousarray(np.asarray(inputs['mask_bin'], np.int32))
    in_maps = []
    for b in range(B):
        m = dict(shared)
        m['q'] = np.ascontiguousarray(q[b])
        m['kv'] = np.ascontiguousarray(kv[b])
        m['mask'] = mask[b]
        in_maps.append(m)
    return in_maps


_NC_CACHE = {}


def get_nc():
    if 'nc' not in _NC_CACHE:
        _NC_CACHE['nc'] = build_nc()
    return _NC_CACHE['nc']


def kernel(**inputs) -> np.ndarray:
    in_maps = _prep_inputs(inputs)
    nc = get_nc()
    res = run_bass_kernel_spmd(nc, in_maps, list(range(B)))
    return np.stack([res.results[b]['out'] for b in range(B)], axis=0)
